# revision 1
# baseline (speedup 1.0000x reference)
"""Trainium2 Bass kernel for the C. elegans Hodgkin-Huxley network simulation.

Strategy
--------
Pure data parallel over the worm/batch axis: 512 worms -> 64 per NeuronCore
across 8 cores.  On each core the state lives transposed ([neuron-slot,
batch]) packed into flat [128, 192] SBUF tiles (3 chunks of 128 slots; the
302 neurons are permuted so all chemical-presynaptic neurons sit in chunk 0
and all gap-junction neurons in chunk 1, duplicating the ~30 neurons in both
sets).  That makes each synaptic matmul a single K<=128 contraction:

    I_syn^T[i,b] = 0.5*Wc^T tanh-act  +  Wg^T V  +  0.985*I*(V, It)

with the leak, the sigmoid affine constants and DT all folded into the
host-prepared weights / external-current tensor.  The ionic currents are
reorganised as iNa+iK = KV(m,h,n)*V + KC(m,h,n) so the V(t)->V(t+1)
critical path is just tanh -> 9 PSUM matmuls -> (ps - KV*V); the gate
update uses Gn = G*((1-B)-A) + A with (1-B) precomputed off the path.
All PSUM accumulation uses a single start=True matmul covering the whole
[128,192] tile (start clears the bank-wide has_written state, so per-chunk
interleaved groups silently drop contributions).  sigmoids are rewritten
via tanh and e2 = e1*exp(-1.5) so one ACT table set (exp/tanh) serves the
whole model.  I/O is staged in blocks of 16 steps (host pre-transposes
I_ext; host post-transposes the voltage trace back).
"""

import os
import numpy as np

# Model constants (must match the reference)
N = 302
DT = 0.05
G_CHEM = 0.1
G_GAP = 0.05
G_NA, E_NA = 120.0, 50.0
G_K, E_K = 36.0, -77.0
G_L, E_L = 0.3, -54.387
V_TH, V_SIG = -20.0, 5.0

B_FULL = 512
T_FULL = 256
N_CORES = 8
BL = B_FULL // N_CORES      # 64 worms per core
NS = 384                    # padded neuron-slot count (3 chunks of 128)
NCH = 3                     # chunks
FW = NCH * BL               # 192 = flat free width of one step's state

_CACHE = {}


def _hh_init_gates():
    """m/h/n at V0=-65 (steady state of the rate equations)."""
    V0 = -65.0
    def vtrap(x, y):
        return x / -np.expm1(-x / y)
    a_m = 0.1 * vtrap(V0 + 40.0, 10.0)
    b_m = 4.0 * np.exp(-(V0 + 65.0) / 18.0)
    a_h = 0.07 * np.exp(-(V0 + 65.0) / 20.0)
    b_h = 1.0 / (1.0 + np.exp(-(V0 + 35.0) / 10.0))
    a_n = 0.01 * vtrap(V0 + 55.0, 10.0)
    b_n = 0.125 * np.exp(-(V0 + 65.0) / 80.0)
    m0 = a_m / (a_m + b_m)
    h0 = a_h / (a_h + b_h)
    n0 = a_n / (a_n + b_n)
    return float(m0), float(h0), float(n0)


def _build_program(T, TC):
    """Build + compile the SPMD Bass program (one core's view)."""
    import concourse.bacc as bacc
    import concourse.mybir as mybir
    import concourse.tile as tile
    from concourse._compat import get_trn_type

    f32 = mybir.dt.float32
    op = mybir.AluOpType
    AF = mybir.ActivationFunctionType

    NB = T // TC
    m0, h0, n0 = _hh_init_gates()

    nc = bacc.Bacc(get_trn_type() or "TRN2", target_bir_lowering=False,
                   debug=False, num_devices=N_CORES)

    it_d = nc.dram_tensor("it_in", [NB, NCH, 128, TC, BL], f32,
                          kind="ExternalInput")
    wc_d = nc.dram_tensor("w_chem", [128, NS], f32, kind="ExternalInput")
    wg_d = nc.dram_tensor("w_gap", [128, NS], f32, kind="ExternalInput")
    wi_d = nc.dram_tensor("w_id", [128, 128], f32, kind="ExternalInput")
    wn_d = nc.dram_tensor("w_negid", [128, 128], f32, kind="ExternalInput")
    v_d = nc.dram_tensor("v_out", [NB, NCH, 128, TC, BL], f32,
                         kind="ExternalOutput")

    # activation immediates
    LN = float(np.log(4.0 * DT))          # b_m multiplier fold
    LAH = float(np.log(0.07 * DT))
    LBN = float(np.log(0.125 * DT))
    BM_B = float(-65.0 / 18.0 + LN)
    SC_K2 = float(np.sqrt(1.8))
    SC_K3 = float(np.sqrt(138.6))
    C_E2 = float(np.exp(-1.5))
    AH_B = float(-65.0 / 20.0 + LAH)
    BN_B = float(-65.0 / 80.0 + LBN)

    with tile.TileContext(nc) as tc_:
        with (
            tc_.tile_pool(name="persist", bufs=1) as pp,
            tc_.tile_pool(name="io", bufs=2) as iop,
            tc_.tile_pool(name="scr", bufs=4) as sp,
            tc_.tile_pool(name="psum", bufs=8, space="PSUM") as psp,
        ):
            # --- persistent tiles -------------------------------------------------
            Wc = pp.tile([128, NS], f32, tag="wc")
            Wg = pp.tile([128, NS], f32, tag="wg")
            Wi = pp.tile([128, 128], f32, tag="wi")
            Wn = pp.tile([128, 128], f32, tag="wn")
            G0 = pp.tile([128, 3 * FW], f32, tag="g0")    # [m | n | h]
            Vinit = pp.tile([128, FW], f32, tag="vinit")
            SingC = pp.tile([128, 2 * FW], f32, tag="singc")

            nc.sync.dma_start(Wc[:], wc_d.ap())
            nc.sync.dma_start(Wg[:], wg_d.ap())
            nc.sync.dma_start(Wi[:], wi_d.ap())
            nc.sync.dma_start(Wn[:], wn_d.ap())
            nc.gpsimd.memset(G0[:, 0:FW], m0)
            nc.gpsimd.memset(G0[:, FW:2 * FW], n0)
            nc.gpsimd.memset(G0[:, 2 * FW:3 * FW], h0)
            nc.gpsimd.memset(Vinit[:], -65.0)
            nc.gpsimd.memset(SingC[:, 0:FW], 0.05)        # A_m at vtrap singularity
            nc.gpsimd.memset(SingC[:, FW:2 * FW], 0.005)  # A_n at vtrap singularity

            # per-partition bias constants for the ACT ops
            bias_vals = [2.0, 1.75, -4.0, -5.5, AH_B, BM_B, BN_B]
            bias_ap = {}
            bias_tile = pp.tile([128, len(bias_vals)], f32, tag="biases")
            for i, bv in enumerate(bias_vals):
                nc.gpsimd.memset(bias_tile[:, i:i + 1], bv)
                bias_ap[bv] = bias_tile[:, i:i + 1]

            Vap = Vinit[:]
            Gap = G0[:]
            for ib in range(NB):
                itb = iop.tile([128, TC * FW], f32, tag="itb")
                outb = iop.tile([128, TC * FW], f32, tag="outb")
                itb_r = itb[:].rearrange("p (t c b) -> p t c b",
                                         t=TC, c=NCH, b=BL)
                for c in range(NCH):
                    nc.sync.dma_start(itb_r[:, :, c, :], it_d.ap()[ib, c])

                for tt in range(TC):
                    # ================= gate-only prep (off the V path) ==========
                    # iNa+iK = KV*V + KC ; KV,KC from m,h,n alone
                    sq = sp.tile([128, 2 * FW], f32, tag="sq")
                    p1 = sp.tile([128, FW], f32, tag="p1")
                    I6 = sp.tile([128, FW], f32, tag="I6")
                    KV = sp.tile([128, FW], f32, tag="KV")
                    KC = sp.tile([128, FW], f32, tag="KC")
                    K2 = sp.tile([128, FW], f32, tag="K2")
                    K3 = sp.tile([128, FW], f32, tag="K3")
                    nc.scalar.square(sq[:], Gap[:, 0:2 * FW])      # [m^2|n^2]
                    n4 = sp.tile([128, FW], f32, tag="n4")
                    nc.scalar.square(n4[:], sq[:, FW:2 * FW])        # n^4
                    nc.gpsimd.tensor_scalar(K2[:], n4[:], 1.8, None, op.mult)
                    nc.gpsimd.tensor_scalar(K3[:], n4[:], 138.6, None, op.mult)
                    p6 = sp.tile([128, FW], f32, tag="p6")
                    nc.gpsimd.tensor_scalar(p6[:], Gap[:, 0:FW], 6.0, None,
                                            op.mult)
                    nc.gpsimd.tensor_tensor(p1[:], p6[:], Gap[:, 2 * FW:3 * FW],
                                            op.mult)                 # 6 m h
                    nc.vector.tensor_tensor(I6[:], p1[:], sq[:, 0:FW],
                                            op.mult)                 # 6 m^3 h
                    nc.vector.tensor_tensor(KV[:], I6[:], K2[:], op.add)
                    nc.vector.scalar_tensor_tensor(KC[:], I6[:], -50.0, K3[:],
                                                   op.mult, op.add)

                    # ================= ACT on V =================================
                    S = sp.tile([128, 6 * FW], f32, tag="S")  # [Am|An|Ah|Bm|Bn|Bh]
                    Ep = sp.tile([128, FW], f32, tag="Ep")    # e1
                    tct = sp.tile([128, BL], f32, tag="tct")
                    tht = sp.tile([128, FW], f32, tag="tht")
                    nc.scalar.activation(Ep[:], Vap, AF.Exp,
                                         bias=bias_ap[-4.0], scale=-0.1)
                    nc.scalar.activation(tct[:], Vap[:, 0:BL], AF.Tanh,
                                         bias=bias_ap[2.0], scale=0.1)
                    nc.scalar.activation(S[:, 2 * FW:3 * FW], Vap, AF.Exp,
                                         bias=bias_ap[AH_B], scale=float(-1 / 20))
                    nc.scalar.activation(S[:, 3 * FW:4 * FW], Vap, AF.Exp,
                                         bias=bias_ap[BM_B], scale=float(-1 / 18))
                    nc.scalar.activation(S[:, 4 * FW:5 * FW], Vap, AF.Exp,
                                         bias=bias_ap[BN_B], scale=float(-1 / 80))
                    nc.scalar.activation(tht[:], Vap, AF.Tanh,
                                         bias=bias_ap[1.75], scale=0.05)

                    # ================= PE: psum accumulation ====================
                    # grouped by stationary weight; per column-range group:
                    # start on the It matmul, stop on the chem matmul
                    ps = psp.tile([128, FW], f32, tag="ps")
                    it0 = tt * FW
                    nc.tensor.matmul(ps[:], Wi[:], itb[:, it0:it0 + FW],
                                     start=True, stop=False)
                    nc.tensor.matmul(ps[:], Wi[:], Vap, start=False, stop=False)
                    nc.tensor.matmul(ps[:], Wn[:], KC[:], start=False, stop=False)
                    for mi in range(NCH):
                        cr = slice(mi * BL, (mi + 1) * BL)
                        nc.tensor.matmul(ps[:, cr], Wg[:, mi * 128:(mi + 1) * 128],
                                         Vap[:, BL:2 * BL], start=False, stop=False)
                    for mi in range(NCH):
                        cr = slice(mi * BL, (mi + 1) * BL)
                        nc.tensor.matmul(ps[:, cr], Wc[:, mi * 128:(mi + 1) * 128],
                                         tct[:], start=False, stop=(mi == NCH - 1),
                                         skip_group_check=True)
                    # ================= voltage update (short V path) ============
                    W3 = sp.tile([128, FW], f32, tag="W3")
                    nc.vector.tensor_tensor(W3[:], KV[:], Vap, op.mult)
                    vout = outb[:, tt * FW:(tt + 1) * FW]
                    nc.vector.tensor_tensor(vout, ps[:], W3[:], op.subtract)

                    # ================= vtrap rates ==============================
                    xp = sp.tile([128, 2 * FW], f32, tag="xp")
                    dd = sp.tile([128, 2 * FW], f32, tag="dd")
                    msk = sp.tile([128, 2 * FW], f32, tag="msk")
                    nc.gpsimd.tensor_scalar(xp[:, 0:FW], Vap, 0.005, 0.2,
                                            op.mult, op.add)
                    nc.gpsimd.tensor_scalar(xp[:, FW:2 * FW], Vap, 0.0005, 0.0275,
                                            op.mult, op.add)
                    nc.vector.tensor_scalar(dd[:, 0:FW], Ep[:], -1.0, 1.0,
                                            op.mult, op.add)
                    nc.vector.tensor_scalar(dd[:, FW:2 * FW], Ep[:], -C_E2, 1.0,
                                            op.mult, op.add)
                    nc.gpsimd.tensor_scalar(msk[:], dd[:], 0.0, None, op.is_equal)
                    rr = sp.tile([128, 2 * FW], f32, tag="rr")
                    nc.vector.reciprocal(rr[:], dd[:])
                    nc.vector.tensor_tensor(S[:, 0:2 * FW], xp[:], rr[:], op.mult)
                    nc.vector.copy_predicated(S[:, 0:2 * FW],
                                              msk[:].bitcast(mybir.dt.int32),
                                              SingC[:])

                    # ============ gates: Gn = G*((1-B)-A) + A ====================
                    Q = sp.tile([128, 3 * FW], f32, tag="Q")
                    R = sp.tile([128, 3 * FW], f32, tag="R")
                    W2 = sp.tile([128, 3 * FW], f32, tag="W2")
                    Gn = sp.tile([128, 3 * FW], f32, tag="Gn")
                    nc.gpsimd.tensor_scalar(S[:, 5 * FW:6 * FW], tht[:],
                                            0.025, 0.025, op.mult, op.add)
                    nc.vector.tensor_scalar(Q[:], S[:, 3 * FW:6 * FW], -1.0, 1.0,
                                            op.mult, op.add)        # 1 - B
                    nc.vector.tensor_tensor(R[:], Q[:], S[:, 0:3 * FW],
                                            op.subtract)            # 1 - B - A
                    nc.vector.tensor_tensor(W2[:], Gap[:, 0:3 * FW], R[:], op.mult)
                    nc.vector.tensor_tensor(Gn[:], W2[:], S[:, 0:3 * FW], op.add)
                    Gap = Gn[:]
                    Vap = vout

                outb_r = outb[:].rearrange("p (t c b) -> p t c b",
                                           t=TC, c=NCH, b=BL)
                for c in range(NCH):
                    nc.sync.dma_start(v_d.ap()[ib, c], outb_r[:, :, c, :])

    nc.compile()
    return nc


def _get_program(T=T_FULL, TC=16):
    key = (T, TC)
    if key not in _CACHE:
        _CACHE[key] = _build_program(T, TC)
    return _CACHE[key]


def _prep_weights(chem, gap):
    """Neuron permutation + folded weight matrices (float32 host prep)."""
    chem = np.asarray(chem, np.float64)
    gap = np.asarray(gap, np.float64)
    gap_eff = gap - np.diag(gap.sum(axis=0))

    p_chem = np.nonzero(np.any(chem != 0.0, axis=1))[0]
    p_gap = np.nonzero(np.any(gap_eff != 0.0, axis=1))[0]
    assert len(p_chem) <= 128, f"chem pre-set {len(p_chem)} > 128"
    assert len(p_gap) <= 128, f"gap set {len(p_gap)} > 128"

    set_c, set_g = set(p_chem.tolist()), set(p_gap.tolist())
    rest = [n for n in range(N) if n not in set_c and n not in set_g]

    chunk0 = list(p_chem)
    fill = [n for n in rest if True]
    while len(chunk0) < 128:
        chunk0.append(fill.pop())
    used = set(chunk0)

    chunk1 = list(p_gap)
    rem = [n for n in range(N) if n not in used and n not in set_g]
    while len(chunk1) < 128 and rem:
        chunk1.append(rem.pop())
    used |= set(chunk1)

    chunk2 = [n for n in range(N) if n not in used]
    assert len(chunk1) <= 128 and len(chunk2) <= 128, \
        (len(chunk0), len(chunk1), len(chunk2))
    slots = np.full(NS, -1, np.int64)
    slots[0:128] = chunk0
    slots[128:128 + len(chunk1)] = chunk1
    slots[256:256 + len(chunk2)] = chunk2

    live = slots >= 0
    # slot_of[n] = first slot holding neuron n
    slot_of = np.full(N, -1, np.int64)
    for s in range(NS - 1, -1, -1):
        if slots[s] >= 0:
            slot_of[slots[s]] = s
    assert (slot_of >= 0).all()

    # per-slot output columns (duplicated for duplicated neurons; 0 for dead)
    col = np.zeros((N, NS), np.float64)
    col[slots[live], np.nonzero(live)[0]] = 1.0   # col[n, s] = 1 iff slots[s]==n

    Wc = (0.5 * DT * G_CHEM) * (chem[np.array(chunk0)] @ col)          # [128, NS]
    Wg = (DT * G_GAP) * (gap_eff[np.array(slots[128:256].clip(min=0))] @ col)
    # zero the rows of Wg whose slot is dead padding
    dead1 = ~live[128:256]
    Wg[dead1] = 0.0
    Wi = (1.0 - DT * G_L) * np.eye(128)

    # per-neuron additive constant: leak offset + chemical sigmoid 0.5-offset
    Cn = DT * G_L * E_L + (0.5 * DT * G_CHEM) * chem.sum(axis=0)       # [N]

    return (Wc.astype(np.float32), Wg.astype(np.float32),
            Wi.astype(np.float32), Cn, slots, slot_of, live)


def kernel(I_ext, chem_weights, gap_weights):
    from concourse.bass_utils import run_bass_kernel_spmd

    I_ext = np.asarray(I_ext, np.float32)
    B, T, Nn = I_ext.shape
    assert (B, T, Nn) == (B_FULL, T_FULL, N)

    Wc, Wg, Wi, Cn, slots, slot_of, live = _prep_weights(
        np.asarray(chem_weights, np.float32), np.asarray(gap_weights, np.float32))

    TC = 16 if T_FULL % 16 == 0 else 1
    NB = T_FULL // TC
    nc = _get_program(T_FULL, TC)

    inv = 1.0 / (1.0 - DT * G_L)
    live_idx = np.nonzero(live)[0]
    nrn = slots[live_idx]

    in_maps = []
    for c in range(N_CORES):
        I_loc = I_ext[c * BL:(c + 1) * BL]                 # [BL, T, N]
        arr = np.zeros((NS, T, BL), np.float32)
        # arr[s, t, b] = (DT*I_loc[b,t,n(s)] + Cn[n(s)]) / (1 - DT*G_L)
        vals = (DT * np.transpose(I_loc, (2, 1, 0))[nrn]
                + Cn[nrn, None, None].astype(np.float64)) * inv
        arr[live_idx] = vals.astype(np.float32)
        # [NS, T, BL] -> [NB, NCH, 128, TC, BL] (block-major staging layout)
        it_blk = np.transpose(
            arr.reshape(NCH, 128, NB, TC, BL), (2, 0, 1, 3, 4))
        in_maps.append({
            "it_in": np.ascontiguousarray(it_blk),
            "w_chem": Wc, "w_gap": Wg, "w_id": Wi,
            "w_negid": -np.eye(128, dtype=np.float32),
        })

    trace = bool(os.environ.get("KERNEL_TRACE"))
    res = run_bass_kernel_spmd(nc, in_maps, list(range(N_CORES)), trace=trace)
    globals()["LAST_RESULTS"] = res

    out = np.empty((B_FULL, T_FULL, N), np.float32)
    for c in range(N_CORES):
        vb = res.results[c]["v_out"].reshape(NB, NCH, 128, TC, BL)
        vd = np.transpose(vb, (1, 2, 0, 3, 4)).reshape(NS, T_FULL, BL)
        out[c * BL:(c + 1) * BL] = np.transpose(vd[slot_of], (2, 1, 0))
    return out

